# revision 14
# baseline (speedup 1.0000x reference)
"""Multi-head attention forward on 8 TRN2 NeuronCores (Bass/Tile).

Sharding: core = b*4 + g  (b in {0,1} batch, g in {0..3} head-group of 4
heads).  Each core computes, for its (b, 4-head group):
  qT/kT = (x@Wq+bq)^T per head in [dh, S] layout (via xT rhs, f32r matmuls),
  V in [S, dh] layout, scores twice (once [q,k] for the softmax-weights
  output, once [k,q] for the attention matmul), softmax without max-
  subtraction (scores are O(6), fp32-safe), attention attT = V^T @ PT with
  the per-head log-denominator folded into the [k,q] score matmul as two
  extra contraction rows (hi/lo f32r), and the output projection with bo
  folded in as a rank-1 matmul.  Host assembles weights and sums the 4
  partial output projections per batch.

All matmul operands are float32r (TF32-like: fp32 with 11-bit mantissa,
RNE-rounded by the PE on the fly) -> 1 cycle/row on the PE at N>=256.
"""

import sys

sys.path.insert(0, "/opt/trn_rl_repo")

import numpy as np
from contextlib import ExitStack

import concourse.bass as bass
from concourse import bacc, mybir
from concourse.tile import TileContext
from concourse.masks import make_identity
from concourse import hw_specs as _hw_specs

# Force Exp/Ln to resolve to the single combined ACT table set so the
# kernel pays one ACT_TABLE_LOAD instead of thrashing between sets.
if not getattr(_hw_specs, "_mha_tables_patched", False):
    _orig_get_tables = _hw_specs.get_activation_tables

    def _patched_get_tables(arch):
        tabs = _orig_get_tables(arch)
        both = {mybir.ActivationFunctionType.Exp,
                mybir.ActivationFunctionType.Ln}
        out = {}
        for name, funcs in tabs.items():
            if (funcs & both) and not (both <= funcs):
                funcs = funcs - both
            out[name] = funcs
        return out

    _hw_specs.get_activation_tables = _patched_get_tables
    bacc.get_activation_tables = _patched_get_tables
    _hw_specs._mha_tables_patched = True

F32 = mybir.dt.float32
F32R = mybir.dt.float32r
AF = mybir.ActivationFunctionType

D = 1024          # model dim
NHC = 4           # heads per core
DH = 64           # head dim
GC = NHC * DH     # 256 projection columns per core
SCALE = 1.0 / 8.0  # 1/sqrt(DH)


def build_nc(S=2048, interleave=True):
    NT = S // 128        # 128-row tiles along seq
    NCK = S // 512       # 512-col chunks along seq
    DC = D // 128        # contraction chunks
    HS = S // 2          # half seq

    nc = bacc.Bacc()
    xT = nc.declare_dram_parameter("xT", [D, S], F32R, isOutput=False)
    wq = nc.declare_dram_parameter("wq", [D, GC], F32R, isOutput=False)
    wk = nc.declare_dram_parameter("wk", [D, GC], F32R, isOutput=False)
    wv = nc.declare_dram_parameter("wv", [D, GC], F32R, isOutput=False)
    wo = nc.declare_dram_parameter("wo", [GC, D], F32R, isOutput=False)
    bq = nc.declare_dram_parameter("bq", [1, GC], F32R, isOutput=False)
    bk = nc.declare_dram_parameter("bk", [1, GC], F32R, isOutput=False)
    bv = nc.declare_dram_parameter("bv", [1, GC], F32R, isOutput=False)
    bo = nc.declare_dram_parameter("bo", [1, D], F32R, isOutput=False)
    ones = nc.declare_dram_parameter("ones", [1, S], F32R, isOutput=False)
    kext = nc.declare_dram_parameter("kext", [2, S], F32R, isOutput=False)
    wpart = nc.declare_dram_parameter("wpart", [NHC, S, S], F32, isOutput=True)
    opart = nc.declare_dram_parameter("opart", [S, D], F32, isOutput=True)

    with TileContext(nc) as tc, ExitStack() as top:
        const = top.enter_context(tc.tile_pool(name="const", bufs=1))
        qkt = top.enter_context(tc.tile_pool(name="qkt", bufs=1))
        vp = top.enter_context(tc.tile_pool(name="vp", bufs=1))
        pp = top.enter_context(tc.tile_pool(name="pp", bufs=2))

        # ---- constants / weights in SBUF
        wq_sb = const.tile([128, DC, GC], F32R)
        wk_sb = const.tile([128, DC, GC], F32R)
        wv_sb = const.tile([128, DC, GC], F32R)
        nc.sync.dma_start(out=wq_sb, in_=wq[:].rearrange("(c p) n -> p c n", p=128))
        nc.sync.dma_start(out=wk_sb, in_=wk[:].rearrange("(c p) n -> p c n", p=128))
        nc.sync.dma_start(out=wv_sb, in_=wv[:].rearrange("(c p) n -> p c n", p=128))
        bq_sb = const.tile([1, GC], F32R)
        bk_sb = const.tile([1, GC], F32R)
        bv_sb = const.tile([1, GC], F32R)
        bo_sb = const.tile([1, D], F32R)
        ones_sb = const.tile([1, S], F32R)
        for t, src in [(bq_sb, bq), (bk_sb, bk), (bv_sb, bv), (bo_sb, bo),
                       (ones_sb, ones)]:
            nc.sync.dma_start(out=t, in_=src[:])
        ident = const.tile([128, 128], F32)
        make_identity(nc, ident[:])

        # per-(head,qtile) row sums of exp(scores), their reciprocals, and
        # the per-head log-denominator rows
        acc_sb = const.tile([128, NHC * NT], F32)
        acc2_sb = const.tile([128, 2 * NHC * NT], F32)
        rec_sb = const.tile([128, NHC * NT], F32)

        # qT/kT per head: [66, S]; rows 0..63 = head dims; rows 64,65 carry
        # the log-denominator hi/lo extension on the q side (written via a
        # partition-shifting SBUF->SBUF DMA) and the constant -8 on the k
        # side (DMA'd from the kext input).
        qT = [qkt.tile([66, S], F32R, name=f"qT{h}", tag=f"qT{h}")
              for h in range(NHC)]
        kT = [qkt.tile([66, S], F32R, name=f"kT{h}", tag=f"kT{h}")
              for h in range(NHC)]
        v_sb = vp.tile([128, NT, GC], F32R)

        q_ctx = ExitStack()
        ps_q = q_ctx.enter_context(
            tc.tile_pool(name="ps_q", bufs=1, space="PSUM"))
        ps_h = q_ctx.enter_context(
            tc.tile_pool(name="ps_h", bufs=1, space="PSUM"))

        def emit_heat(n):
            # dummy matmuls that keep the PE's HAM activity monitor busy so
            # the 2.4 GHz clock state survives ACT-bound stretches
            for _ in range(n):
                ht = ps_h.tile([128, 512], F32, tag="heat")
                nc.tensor.matmul(ht[:], wq_sb[:, 0, 0:128], wq_sb[:, 0:2, :],
                                 start=True, stop=True)

        xt_ctx = ExitStack()
        xtp = xt_ctx.enter_context(tc.tile_pool(name="xtp", bufs=1))
        xT_sb = xtp.tile([128, DC, S], F32R)
        for c in range(DC):
            nc.sync.dma_start(out=xT_sb[:, c, :],
                              in_=xT[c * 128:(c + 1) * 128, :])

        proj_ctx = ExitStack()
        ps_qk = proj_ctx.enter_context(
            tc.tile_pool(name="ps_qk", bufs=2, space="PSUM"))
        ps_v = proj_ctx.enter_context(
            tc.tile_pool(name="ps_v", bufs=2, space="PSUM"))

        def emit_proj_head(h):
            for dst, w_sb, b_sb in ((qT[h], wq_sb, bq_sb), (kT[h], wk_sb, bk_sb)):
                for nck in range(NCK):
                    ps = ps_qk.tile([64, 512], F32, tag="qk")
                    for c in range(DC):
                        nc.tensor.matmul(
                            ps[:], w_sb[:, c, h * 64:(h + 1) * 64],
                            xT_sb[:, c, nck * 512:(nck + 1) * 512],
                            start=(c == 0), stop=False)
                    nc.tensor.matmul(
                        ps[:], b_sb[0:1, h * 64:(h + 1) * 64],
                        ones_sb[0:1, nck * 512:(nck + 1) * 512],
                        start=False, stop=True)
                    nc.vector.tensor_copy(
                        dst[0:64, nck * 512:(nck + 1) * 512], ps[:])
            nc.sync.dma_start(out=kT[h][64:66, :], in_=kext[:])

        def emit_proj_v():
            for st in range(NT):
                ps = ps_v.tile([128, GC], F32, tag="v")
                for c in range(DC):
                    nc.tensor.matmul(
                        ps[:], xT_sb[:, c, st * 128:(st + 1) * 128],
                        wv_sb[:, c, :], start=(c == 0), stop=False)
                nc.tensor.matmul(
                    ps[:], ones_sb[0:1, st * 128:(st + 1) * 128], bv_sb[:],
                    start=False, stop=True)
                nc.vector.tensor_copy(v_sb[:, st, :], ps[:])

        def emit_q_tile(h, qt):
            col = h * NT + qt
            pes = []
            for kh in range(2):
                sc = ps_q.tile([128, HS], F32, tag="sc")
                for nck in range(HS // 512):
                    k0 = kh * HS + nck * 512
                    nc.tensor.matmul(
                        sc[:, nck * 512:(nck + 1) * 512],
                        qT[h][0:64, qt * 128:(qt + 1) * 128],
                        kT[h][0:64, k0:k0 + 512],
                        start=True, stop=True)
                pe = pp.tile([128, HS], F32, tag="pe", bufs=4)
                nc.scalar.activation(pe[:], sc[:], AF.Exp, scale=SCALE,
                                     accum_out=acc2_sb[:, 2 * col + kh:
                                                       2 * col + kh + 1])
                pes.append(pe)
            nc.vector.tensor_add(acc_sb[:, col:col + 1],
                                 acc2_sb[:, 2 * col:2 * col + 1],
                                 acc2_sb[:, 2 * col + 1:2 * col + 2])
            nc.vector.reciprocal(rec_sb[:, col:col + 1], acc_sb[:, col:col + 1])
            for kh in range(2):
                pe = pes[kh]
                nc.vector.tensor_scalar_mul(pe[:], pe[:], rec_sb[:, col:col + 1])
                nc.gpsimd.dma_start(
                    out=wpart[h, qt * 128:(qt + 1) * 128, kh * HS:(kh + 1) * HS],
                    in_=pe[:])

        att_ctx = ExitStack()
        t_ctx = ExitStack()
        ps_t = ps_ar = attp = ptp = None
        attT = []
        lrow_sb = None
        lext_sb = None

        def open_t_pools():
            nonlocal ps_t, ps_ar, attp, ptp, attT, lrow_sb, lext_sb
            attp = att_ctx.enter_context(tc.tile_pool(name="attp", bufs=1))
            for h in range(NHC):
                attT.append(attp.tile([64, S], F32R, name=f"attT{h}",
                                      tag=f"attT{h}"))
            lrow_sb = attp.tile([1, S], F32)
            lext_sb = attp.tile([33, S], F32R)
            ptp = t_ctx.enter_context(tc.tile_pool(name="ptp", bufs=3))
            ps_t = t_ctx.enter_context(
                tc.tile_pool(name="ps_t", bufs=1, space="PSUM"))
            ps_ar = t_ctx.enter_context(
                tc.tile_pool(name="ps_ar", bufs=1, space="PSUM"))

        def emit_rows(h):
            # accum columns [128,1] -> [1,S] log-denominator row, then the
            # hi/lo f32r extension rows of qT[h]
            for piece in range(NT // 4):
                rp = ps_ar.tile([1, 512], F32, tag="ar")
                for j in range(4):
                    qt = piece * 4 + j
                    nc.tensor.matmul(
                        rp[0:1, j * 128:(j + 1) * 128],
                        acc_sb[:, h * NT + qt:h * NT + qt + 1], ident[:],
                        is_transpose=True, start=(j == 0), stop=(j == 3))
                nc.scalar.activation(
                    lrow_sb[0:1, piece * 512:(piece + 1) * 512], rp[:],
                    AF.Ln)
            nc.vector.tensor_copy(lext_sb[0:1, :], lrow_sb[0:1, :])
            nc.vector.tensor_sub(lext_sb[32:33, :], lrow_sb[0:1, :],
                                 lext_sb[0:1, :])
            nc.sync.dma_start(out=qT[h][64:66, :], in_=lext_sb[::32, :])

        def emit_t_tile(h, half, kt):
            q0 = half * HS
            sct = ps_t.tile([128, HS], F32, tag="sct")
            for ncq in range(HS // 512):
                nc.tensor.matmul(
                    sct[:, ncq * 512:(ncq + 1) * 512],
                    kT[h][:, kt * 128:(kt + 1) * 128],
                    qT[h][:, q0 + ncq * 512:q0 + (ncq + 1) * 512],
                    start=True, stop=True)
            pt = ptp.tile([128, HS], F32R, tag="pt")
            nc.scalar.activation(pt[:], sct[:], AF.Exp, scale=SCALE)
            return pt

        # ------------------------------------------------------------------
        # Emission schedule
        # ------------------------------------------------------------------
        emit_proj_head(0)

        def q_tiles_of(h):
            for qt in range(NT):
                yield (h, qt)

        # Q(h0) + remaining projections
        for qt in range(NT):
            emit_q_tile(0, qt)
        for h in range(1, NHC):
            emit_proj_head(h)
        emit_proj_v()
        proj_ctx.close()
        xt_ctx.close()
        open_t_pools()

        for h in range(NHC):
            emit_rows(h)
            emit_heat(4)
            q_iter = q_tiles_of(h + 1) if h + 1 < NHC else iter(())
            for half in range(2):
                # T tiles with att accumulation; interleave next head's Q
                q0 = half * HS
                at = ps_ar.tile([64, HS], F32, tag="ar")
                for kt in range(NT):
                    if interleave and kt % 2 == 0:
                        nxt = next(q_iter, None)
                        if nxt is not None:
                            emit_q_tile(*nxt)
                    pt = emit_t_tile(h, half, kt)
                    for ncq in range(HS // 512):
                        nc.tensor.matmul(
                            at[:, ncq * 512:(ncq + 1) * 512],
                            v_sb[:, kt, h * 64:(h + 1) * 64],
                            pt[:, ncq * 512:(ncq + 1) * 512],
                            start=(kt == 0), stop=(kt == NT - 1))
                    emit_heat(2)
                nc.vector.tensor_copy(attT[h][0:64, q0:q0 + HS], at[:])
            for nxt in q_iter:
                emit_q_tile(*nxt)
                emit_heat(2)

        ps_ar = None
        ps_t = None
        # close ptp + PSUM t pools; attp (attT tiles) stays for outproj
        t_ctx.close()

        out_ctx = ExitStack()
        wop = out_ctx.enter_context(tc.tile_pool(name="wop", bufs=1))
        wo_sb = wop.tile([64, NHC, D], F32R)
        nc.sync.dma_start(out=wo_sb, in_=wo[:].rearrange("(h p) n -> p h n", p=64))
        outp = out_ctx.enter_context(tc.tile_pool(name="outp", bufs=3))
        ps_o = out_ctx.enter_context(
            tc.tile_pool(name="ps_o", bufs=2, space="PSUM"))
        for st in range(NT):
            for nh in range(D // 512):
                po = ps_o.tile([128, 512], F32, tag="po")
                for h in range(NHC):
                    nc.tensor.matmul(
                        po[:],
                        attT[h][0:64, st * 128:(st + 1) * 128],
                        wo_sb[0:64, h, nh * 512:(nh + 1) * 512],
                        start=(h == 0), stop=False)
                nc.tensor.matmul(
                    po[:], ones_sb[0:1, st * 128:(st + 1) * 128],
                    bo_sb[0:1, nh * 512:(nh + 1) * 512],
                    start=False, stop=True)
                ot = outp.tile([128, 512], F32, tag="ot")
                nc.vector.tensor_copy(ot[:], po[:])
                nc.sync.dma_start(
                    out=opart[st * 128:(st + 1) * 128, nh * 512:(nh + 1) * 512],
                    in_=ot[:])
        out_ctx.close()
        att_ctx.close()
        q_ctx.close()

    return nc


_NC_CACHE = {}


def _get_nc(S):
    if S not in _NC_CACHE:
        nc = build_nc(S)
        nc.finalize()
        _NC_CACHE[S] = nc
    return _NC_CACHE[S]


def _shard_inputs(x, wq, bq, wk, bk, wv, bv, wo, bo):
    B, S, d = x.shape
    assert d == D
    c = np.ascontiguousarray
    in_maps = []
    ones = np.ones((1, S), np.float32)
    kext = np.full((2, S), -8.0, np.float32)
    for core in range(8):
        b, g = divmod(core, 4)
        sl = slice(g * GC, (g + 1) * GC)
        in_maps.append({
            "xT": c(x[b].T),
            "wq": c(wq[:, sl]), "wk": c(wk[:, sl]), "wv": c(wv[:, sl]),
            "wo": c(wo[sl, :]),
            "bq": c(bq[sl][None, :]), "bk": c(bk[sl][None, :]),
            "bv": c(bv[sl][None, :]),
            "bo": (bo[None, :].copy() if g == 0
                   else np.zeros((1, D), np.float32)),
            "ones": ones,
            "kext": kext,
        })
    return in_maps


def kernel(x, wq, bq, wk, bk, wv, bv, wo, bo, _run_kwargs=None):
    x = np.asarray(x, np.float32)
    B, S, d = x.shape
    nc = _get_nc(S)
    in_maps = _shard_inputs(np.asarray(x, np.float32),
                            np.asarray(wq, np.float32),
                            np.asarray(bq, np.float32),
                            np.asarray(wk, np.float32),
                            np.asarray(bk, np.float32),
                            np.asarray(wv, np.float32),
                            np.asarray(bv, np.float32),
                            np.asarray(wo, np.float32),
                            np.asarray(bo, np.float32))
    from concourse.bass_utils import run_bass_kernel_spmd
    res = run_bass_kernel_spmd(nc, in_maps, list(range(8)),
                               **(_run_kwargs or {}))
    results = res.results
    NH = 16
    weights = np.empty((B, NH, S, S), np.float32)
    out = np.zeros((B, S, D), np.float32)
    for core in range(8):
        b, g = divmod(core, 4)
        weights[b, g * NHC:(g + 1) * NHC] = results[core]["wpart"]
        out[b] += results[core]["opart"]
    if _run_kwargs is not None:
        kernel.last_result = res
    return out, weights


# revision 15
# speedup vs baseline: 1.2692x; 1.2692x over previous
"""Multi-head attention forward on 8 TRN2 NeuronCores (Bass/Tile).

Sharding: core = b*4 + g  (b in {0,1} batch, g in {0..3} head-group of 4
heads).  Each core computes, for its (b, 4-head group):
  qT/kT = (x@Wq+bq)^T per head in [dh, S] layout (via xT rhs, f32r matmuls),
  V in [S, dh] layout, scores twice (once [q,k] for the softmax-weights
  output, once [k,q] for the attention matmul), softmax without max-
  subtraction (scores are O(6), fp32-safe), attention attT = V^T @ PT with
  the per-head log-denominator folded into the [k,q] score matmul as two
  extra contraction rows (hi/lo f32r), and the output projection with bo
  folded in as a rank-1 matmul.  Host assembles weights and sums the 4
  partial output projections per batch.

All matmul operands are float32r (TF32-like: fp32 with 11-bit mantissa,
RNE-rounded by the PE on the fly) -> 1 cycle/row on the PE at N>=256.
"""

import sys

sys.path.insert(0, "/opt/trn_rl_repo")

import numpy as np
from contextlib import ExitStack

import concourse.bass as bass
from concourse import bacc, mybir
from concourse.tile import TileContext
from concourse.masks import make_identity
from concourse import hw_specs as _hw_specs

# Force Exp/Ln to resolve to the single combined ACT table set so the
# kernel pays one ACT_TABLE_LOAD instead of thrashing between sets.
if not getattr(_hw_specs, "_mha_tables_patched", False):
    _orig_get_tables = _hw_specs.get_activation_tables

    def _patched_get_tables(arch):
        tabs = _orig_get_tables(arch)
        both = {mybir.ActivationFunctionType.Exp,
                mybir.ActivationFunctionType.Ln}
        out = {}
        for name, funcs in tabs.items():
            if (funcs & both) and not (both <= funcs):
                funcs = funcs - both
            out[name] = funcs
        return out

    _hw_specs.get_activation_tables = _patched_get_tables
    bacc.get_activation_tables = _patched_get_tables
    _hw_specs._mha_tables_patched = True

F32 = mybir.dt.float32
F32R = mybir.dt.float32r
AF = mybir.ActivationFunctionType

D = 1024          # model dim
NHC = 4           # heads per core
DH = 64           # head dim
GC = NHC * DH     # 256 projection columns per core
SCALE = 1.0 / 8.0  # 1/sqrt(DH)


def build_nc(S=2048, interleave=True):
    NT = S // 128        # 128-row tiles along seq
    NCK = S // 512       # 512-col chunks along seq
    DC = D // 128        # contraction chunks
    HS = S // 2          # half seq

    nc = bacc.Bacc()
    xT = nc.declare_dram_parameter("xT", [D, S], F32R, isOutput=False)
    wq = nc.declare_dram_parameter("wq", [D, GC], F32R, isOutput=False)
    wk = nc.declare_dram_parameter("wk", [D, GC], F32R, isOutput=False)
    wv = nc.declare_dram_parameter("wv", [D, GC], F32R, isOutput=False)
    wo = nc.declare_dram_parameter("wo", [GC, D], F32R, isOutput=False)
    bq = nc.declare_dram_parameter("bq", [1, GC], F32R, isOutput=False)
    bk = nc.declare_dram_parameter("bk", [1, GC], F32R, isOutput=False)
    bv = nc.declare_dram_parameter("bv", [1, GC], F32R, isOutput=False)
    bo = nc.declare_dram_parameter("bo", [1, D], F32R, isOutput=False)
    ones = nc.declare_dram_parameter("ones", [1, S], F32R, isOutput=False)
    kext = nc.declare_dram_parameter("kext", [2, S], F32R, isOutput=False)
    wpart = nc.declare_dram_parameter("wpart", [NHC, S, S], F32, isOutput=True)
    opart = nc.declare_dram_parameter("opart", [S, D], F32, isOutput=True)

    with TileContext(nc) as tc, ExitStack() as top:
        const = top.enter_context(tc.tile_pool(name="const", bufs=1))
        qkt = top.enter_context(tc.tile_pool(name="qkt", bufs=1))
        vp = top.enter_context(tc.tile_pool(name="vp", bufs=1))
        pp = top.enter_context(tc.tile_pool(name="pp", bufs=2))

        # ---- constants / weights in SBUF
        wq_sb = const.tile([128, DC, GC], F32R)
        wk_sb = const.tile([128, DC, GC], F32R)
        wv_sb = const.tile([128, DC, GC], F32R)
        nc.sync.dma_start(out=wq_sb, in_=wq[:].rearrange("(c p) n -> p c n", p=128))
        nc.sync.dma_start(out=wk_sb, in_=wk[:].rearrange("(c p) n -> p c n", p=128))
        nc.sync.dma_start(out=wv_sb, in_=wv[:].rearrange("(c p) n -> p c n", p=128))
        bq_sb = const.tile([1, GC], F32R)
        bk_sb = const.tile([1, GC], F32R)
        bv_sb = const.tile([1, GC], F32R)
        bo_sb = const.tile([1, D], F32R)
        ones_sb = const.tile([1, S], F32R)
        for t, src in [(bq_sb, bq), (bk_sb, bk), (bv_sb, bv), (bo_sb, bo),
                       (ones_sb, ones)]:
            nc.sync.dma_start(out=t, in_=src[:])
        ident = const.tile([128, 128], F32)
        make_identity(nc, ident[:])

        # per-(head,qtile) row sums of exp(scores), their reciprocals, and
        # the per-head log-denominator rows
        acc_sb = const.tile([128, NHC * NT], F32)
        acc2_sb = const.tile([128, 2 * NHC * NT], F32)
        rec_sb = const.tile([128, NHC * NT], F32)

        # qT/kT per head: [66, S]; rows 0..63 = head dims; rows 64,65 carry
        # the log-denominator hi/lo extension on the q side (written via a
        # partition-shifting SBUF->SBUF DMA) and the constant -8 on the k
        # side (DMA'd from the kext input).
        qT = [qkt.tile([66, S], F32R, name=f"qT{h}", tag=f"qT{h}")
              for h in range(NHC)]
        kT = [qkt.tile([66, S], F32R, name=f"kT{h}", tag=f"kT{h}")
              for h in range(NHC)]
        v_sb = vp.tile([128, NT, GC], F32R)

        q_ctx = ExitStack()
        ps_q = q_ctx.enter_context(
            tc.tile_pool(name="ps_q", bufs=1, space="PSUM"))

        def emit_heat(n):
            pass

        xt_ctx = ExitStack()
        xtp = xt_ctx.enter_context(tc.tile_pool(name="xtp", bufs=1))
        xT_sb = xtp.tile([128, DC, S], F32R)
        for c in range(DC):
            nc.sync.dma_start(out=xT_sb[:, c, :],
                              in_=xT[c * 128:(c + 1) * 128, :])

        proj_ctx = ExitStack()
        ps_qk = proj_ctx.enter_context(
            tc.tile_pool(name="ps_qk", bufs=2, space="PSUM"))
        ps_v = proj_ctx.enter_context(
            tc.tile_pool(name="ps_v", bufs=2, space="PSUM"))

        def emit_proj_head(h):
            for dst, w_sb, b_sb in ((qT[h], wq_sb, bq_sb), (kT[h], wk_sb, bk_sb)):
                for nck in range(NCK):
                    ps = ps_qk.tile([64, 512], F32, tag="qk")
                    for c in range(DC):
                        nc.tensor.matmul(
                            ps[:], w_sb[:, c, h * 64:(h + 1) * 64],
                            xT_sb[:, c, nck * 512:(nck + 1) * 512],
                            start=(c == 0), stop=False)
                    nc.tensor.matmul(
                        ps[:], b_sb[0:1, h * 64:(h + 1) * 64],
                        ones_sb[0:1, nck * 512:(nck + 1) * 512],
                        start=False, stop=True)
                    nc.vector.tensor_copy(
                        dst[0:64, nck * 512:(nck + 1) * 512], ps[:])
            nc.sync.dma_start(out=kT[h][64:66, :], in_=kext[:])

        def emit_proj_v():
            for st in range(NT):
                ps = ps_v.tile([128, GC], F32, tag="v")
                for c in range(DC):
                    nc.tensor.matmul(
                        ps[:], xT_sb[:, c, st * 128:(st + 1) * 128],
                        wv_sb[:, c, :], start=(c == 0), stop=False)
                nc.tensor.matmul(
                    ps[:], ones_sb[0:1, st * 128:(st + 1) * 128], bv_sb[:],
                    start=False, stop=True)
                nc.vector.tensor_copy(v_sb[:, st, :], ps[:])

        def emit_q_tile(h, qt):
            col = h * NT + qt
            pes = []
            for kh in range(2):
                sc = ps_q.tile([128, HS], F32, tag="sc")
                for nck in range(HS // 512):
                    k0 = kh * HS + nck * 512
                    nc.tensor.matmul(
                        sc[:, nck * 512:(nck + 1) * 512],
                        qT[h][0:64, qt * 128:(qt + 1) * 128],
                        kT[h][0:64, k0:k0 + 512],
                        start=True, stop=True)
                pe = pp.tile([128, HS], F32, tag="pe", bufs=4)
                nc.scalar.activation(pe[:], sc[:], AF.Exp, scale=SCALE,
                                     accum_out=acc2_sb[:, 2 * col + kh:
                                                       2 * col + kh + 1])
                pes.append(pe)
            nc.vector.tensor_add(acc_sb[:, col:col + 1],
                                 acc2_sb[:, 2 * col:2 * col + 1],
                                 acc2_sb[:, 2 * col + 1:2 * col + 2])
            nc.vector.reciprocal(rec_sb[:, col:col + 1], acc_sb[:, col:col + 1])
            for kh in range(2):
                pe = pes[kh]
                nc.vector.tensor_scalar_mul(pe[:], pe[:], rec_sb[:, col:col + 1])
                nc.gpsimd.dma_start(
                    out=wpart[h, qt * 128:(qt + 1) * 128, kh * HS:(kh + 1) * HS],
                    in_=pe[:])

        att_ctx = ExitStack()
        t_ctx = ExitStack()
        ps_t = ps_ar = attp = ptp = None
        attT = []
        lrow_sb = None
        lext_sb = None

        def open_t_pools():
            nonlocal ps_t, ps_ar, attp, ptp, attT, lrow_sb, lext_sb
            attp = att_ctx.enter_context(tc.tile_pool(name="attp", bufs=1))
            for h in range(NHC):
                attT.append(attp.tile([64, S], F32R, name=f"attT{h}",
                                      tag=f"attT{h}"))
            lrow_sb = attp.tile([1, S], F32)
            lext_sb = attp.tile([33, S], F32R)
            ptp = t_ctx.enter_context(tc.tile_pool(name="ptp", bufs=3))
            ps_t = t_ctx.enter_context(
                tc.tile_pool(name="ps_t", bufs=2, space="PSUM"))
            ps_ar = t_ctx.enter_context(
                tc.tile_pool(name="ps_ar", bufs=1, space="PSUM"))

        def emit_rows(h):
            # accum columns [128,1] -> [1,S] log-denominator row, then the
            # hi/lo f32r extension rows of qT[h]
            for piece in range(NT // 4):
                rp = ps_q.tile([1, 512], F32, tag="sc")
                for j in range(4):
                    qt = piece * 4 + j
                    nc.tensor.matmul(
                        rp[0:1, j * 128:(j + 1) * 128],
                        acc_sb[:, h * NT + qt:h * NT + qt + 1], ident[:],
                        is_transpose=True, start=(j == 0), stop=(j == 3))
                nc.scalar.activation(
                    lrow_sb[0:1, piece * 512:(piece + 1) * 512], rp[:],
                    AF.Ln)
            nc.vector.tensor_copy(lext_sb[0:1, :], lrow_sb[0:1, :])
            nc.vector.tensor_sub(lext_sb[32:33, :], lrow_sb[0:1, :],
                                 lext_sb[0:1, :])
            nc.sync.dma_start(out=qT[h][64:66, :], in_=lext_sb[::32, :])

        def emit_t_tile(h, half, kt):
            q0 = half * HS
            sct = ps_t.tile([128, HS], F32, tag="sct")
            for ncq in range(HS // 512):
                nc.tensor.matmul(
                    sct[:, ncq * 512:(ncq + 1) * 512],
                    kT[h][:, kt * 128:(kt + 1) * 128],
                    qT[h][:, q0 + ncq * 512:q0 + (ncq + 1) * 512],
                    start=True, stop=True)
            pt = ptp.tile([128, HS], F32R, tag="pt")
            nc.scalar.activation(pt[:], sct[:], AF.Exp, scale=SCALE)
            return pt

        # ------------------------------------------------------------------
        # Emission schedule
        # ------------------------------------------------------------------
        emit_proj_head(0)

        def q_tiles_of(h):
            for qt in range(NT):
                yield (h, qt)

        # Q(h0) + remaining projections
        for qt in range(NT):
            emit_q_tile(0, qt)
        for h in range(1, NHC):
            emit_proj_head(h)
        emit_proj_v()
        proj_ctx.close()
        xt_ctx.close()
        open_t_pools()
        emit_rows(0)

        for h in range(NHC):
            q_iter = q_tiles_of(h + 1) if h + 1 < NHC else iter(())
            rows_done = h + 1 >= NHC
            for half in range(2):
                # T tiles with att accumulation; interleave next head's Q
                q0 = half * HS
                at = ps_ar.tile([64, HS], F32, tag="ar")
                for kt in range(NT):
                    if interleave:
                        nxt = next(q_iter, None)
                        if nxt is not None:
                            emit_q_tile(*nxt)
                        elif not rows_done:
                            emit_rows(h + 1)
                            rows_done = True
                    pt = emit_t_tile(h, half, kt)
                    for ncq in range(HS // 512):
                        nc.tensor.matmul(
                            at[:, ncq * 512:(ncq + 1) * 512],
                            v_sb[:, kt, h * 64:(h + 1) * 64],
                            pt[:, ncq * 512:(ncq + 1) * 512],
                            start=(kt == 0), stop=(kt == NT - 1))
                nc.vector.tensor_copy(attT[h][0:64, q0:q0 + HS], at[:])
            for nxt in q_iter:
                emit_q_tile(*nxt)
            if not rows_done:
                emit_rows(h + 1)

        ps_ar = None
        ps_t = None
        # close ptp + PSUM t pools; attp (attT tiles) stays for outproj
        t_ctx.close()

        out_ctx = ExitStack()
        wop = out_ctx.enter_context(tc.tile_pool(name="wop", bufs=1))
        wo_sb = wop.tile([64, NHC, D], F32R)
        nc.sync.dma_start(out=wo_sb, in_=wo[:].rearrange("(h p) n -> p h n", p=64))
        outp = out_ctx.enter_context(tc.tile_pool(name="outp", bufs=3))
        ps_o = out_ctx.enter_context(
            tc.tile_pool(name="ps_o", bufs=2, space="PSUM"))
        for st in range(NT):
            for nh in range(D // 512):
                po = ps_o.tile([128, 512], F32, tag="po")
                for h in range(NHC):
                    nc.tensor.matmul(
                        po[:],
                        attT[h][0:64, st * 128:(st + 1) * 128],
                        wo_sb[0:64, h, nh * 512:(nh + 1) * 512],
                        start=(h == 0), stop=False)
                nc.tensor.matmul(
                    po[:], ones_sb[0:1, st * 128:(st + 1) * 128],
                    bo_sb[0:1, nh * 512:(nh + 1) * 512],
                    start=False, stop=True)
                ot = outp.tile([128, 512], F32, tag="ot")
                nc.vector.tensor_copy(ot[:], po[:])
                nc.sync.dma_start(
                    out=opart[st * 128:(st + 1) * 128, nh * 512:(nh + 1) * 512],
                    in_=ot[:])
        out_ctx.close()
        att_ctx.close()
        q_ctx.close()

    return nc


_NC_CACHE = {}


def _get_nc(S):
    if S not in _NC_CACHE:
        nc = build_nc(S)
        nc.finalize()
        _NC_CACHE[S] = nc
    return _NC_CACHE[S]


def _shard_inputs(x, wq, bq, wk, bk, wv, bv, wo, bo):
    B, S, d = x.shape
    assert d == D
    c = np.ascontiguousarray
    in_maps = []
    ones = np.ones((1, S), np.float32)
    kext = np.full((2, S), -8.0, np.float32)
    for core in range(8):
        b, g = divmod(core, 4)
        sl = slice(g * GC, (g + 1) * GC)
        in_maps.append({
            "xT": c(x[b].T),
            "wq": c(wq[:, sl]), "wk": c(wk[:, sl]), "wv": c(wv[:, sl]),
            "wo": c(wo[sl, :]),
            "bq": c(bq[sl][None, :]), "bk": c(bk[sl][None, :]),
            "bv": c(bv[sl][None, :]),
            "bo": (bo[None, :].copy() if g == 0
                   else np.zeros((1, D), np.float32)),
            "ones": ones,
            "kext": kext,
        })
    return in_maps


def kernel(x, wq, bq, wk, bk, wv, bv, wo, bo, _run_kwargs=None):
    x = np.asarray(x, np.float32)
    B, S, d = x.shape
    nc = _get_nc(S)
    in_maps = _shard_inputs(np.asarray(x, np.float32),
                            np.asarray(wq, np.float32),
                            np.asarray(bq, np.float32),
                            np.asarray(wk, np.float32),
                            np.asarray(bk, np.float32),
                            np.asarray(wv, np.float32),
                            np.asarray(bv, np.float32),
                            np.asarray(wo, np.float32),
                            np.asarray(bo, np.float32))
    from concourse.bass_utils import run_bass_kernel_spmd
    res = run_bass_kernel_spmd(nc, in_maps, list(range(8)),
                               **(_run_kwargs or {}))
    results = res.results
    NH = 16
    weights = np.empty((B, NH, S, S), np.float32)
    out = np.zeros((B, S, D), np.float32)
    for core in range(8):
        b, g = divmod(core, 4)
        weights[b, g * NHC:(g + 1) * NHC] = results[core]["wpart"]
        out[b] += results[core]["opart"]
    if _run_kwargs is not None:
        kernel.last_result = res
    return out, weights
